# revision 1
# baseline (speedup 1.0000x reference)
"""Trainium2 Bass kernel for nn_AxisSimplestSpline (PE-accumulated clamp basis).

Math (per batch b, axis a):
  f = A^T raw; g = (f - mins_a)/dx_a in [0,17)

est_a(g) = Y0_a + sum_{k=0..16} s_{a,k} * clamp01(g_a - k),  g = (f-mins)/dx
out[c]   = [sum_a pinv[a,c] Y0_a]          (folded into final ACT-copy bias)
         + sum_k matmul(Wk_fp16, C_k_fp16) (accumulated in output PSUM, fp32)

fp16 features are exact where it matters: C in [0,1] (err <= 2^-12), and
values >= 1 clamp exactly.  Knot terms + output projection fused into 17
fp16 matmuls at 1 cycle/row, accumulated in the output PSUM (start/stop
flags; DVE-produced knots emitted first so the in-order PE never waits on
the slower ACT stream).  Engine split: ACT 11 relus (k=1..11); DVE: g,
boundary knots k=0/16 as single ops (exact by the g-range guarantee),
4 dual-op knots, 15 min-ops at 4x mode, and the output-PSUM drain.
Input projection: host-split fp16 hi/lo raw, 2 matmuls (Ah*h, then
[Al;Ah] against stacked [hi;lo] — error ~2^-22).  Measured: 888 us/core,
rel err 4.4e-4; engines converged (PE ~990 busy, ACT ~894, DVE ~854).
"""

import sys

sys.path.insert(0, "/opt/trn_rl_repo")

import numpy as np

import concourse.bacc as bacc
import concourse.mybir as mybir
import concourse.tile as tile
from concourse.bass_utils import run_bass_kernel_spmd

F32 = mybir.dt.float32
F16 = mybir.dt.float16
EPS = 1e-4
B, C, H, W = 8, 3, 1024, 1024
HW = H * W
NA, K = 8, 16
NK = K + 1
J = 16
NJ = HW // J
FREE = 1024
NSUP = NJ // FREE

ACT_SET = set(range(1, 12))  # ACT relu -> fp16, DVE min @4x; k=0,16 are single-op

_NC_CACHE = {}


def _build_nc():
    nc = bacc.Bacc(None, target_bir_lowering=False, debug=False)
    rawh_t = nc.dram_tensor("rawh", [C, HW], F16, kind="ExternalInput")
    rawl_t = nc.dram_tensor("rawl", [C, HW], F16, kind="ExternalInput")
    # par cols: 0:17 act bias (-mins/dx - k), 17 inv_dx, 18 neg mins/dx
    par_t = nc.dram_tensor("par", [128, 19], F32, kind="ExternalInput")
    wfh_t = nc.dram_tensor("wfh", [C * J, 128], F16, kind="ExternalInput")
    wf2_t = nc.dram_tensor("wf2", [2 * C * J, 128], F16, kind="ExternalInput")
    wks_t = nc.dram_tensor("wks", [128, NK * C * J], F16, kind="ExternalInput")
    bout_t = nc.dram_tensor("bout", [C * J, 1], F32, kind="ExternalInput")
    out_t = nc.dram_tensor("out", [C, HW], F32, kind="ExternalOutput")

    Relu = mybir.ActivationFunctionType.Relu
    Ident = mybir.ActivationFunctionType.Identity
    mult = mybir.AluOpType.mult
    add = mybir.AluOpType.add
    mn = mybir.AluOpType.min
    mx = mybir.AluOpType.max
    sub = mybir.AluOpType.subtract

    with tile.TileContext(nc) as tc:
        with (
            tc.tile_pool(name="const", bufs=1) as cpool,
            tc.tile_pool(name="io", bufs=4) as iopool,
            tc.tile_pool(name="gg", bufs=3) as gpool,
            tc.tile_pool(name="rr", bufs=10) as rpool,
            tc.tile_pool(name="cc", bufs=16) as ccpool,
            tc.tile_pool(name="ob", bufs=3) as obpool,
            tc.tile_pool(name="pf", bufs=2, space="PSUM") as pfpool,
            tc.tile_pool(name="po", bufs=2, space="PSUM") as popool,
        ):
            pT = cpool.tile([128, 19], F32)
            nc.sync.dma_start(out=pT[:], in_=par_t[:])
            wfh = cpool.tile([C * J, 128], F16)
            nc.sync.dma_start(out=wfh[:], in_=wfh_t[:])
            wf2 = cpool.tile([2 * C * J, 128], F16)
            nc.sync.dma_start(out=wf2[:], in_=wf2_t[:])
            wks = cpool.tile([128, NK * C * J], F16)
            nc.sync.dma_start(out=wks[:], in_=wks_t[:])
            bout = cpool.tile([C * J, 1], F32)
            nc.sync.dma_start(out=bout[:], in_=bout_t[:])

            rawh_v = rawh_t.ap().rearrange("c (j n) -> (c j) n", j=J)
            rawl_v = rawl_t.ap().rearrange("c (j n) -> (c j) n", j=J)
            out_v = out_t.ap().rearrange("c (j n) -> (c j) n", j=J)
            NCH = FREE // 512

            for s in range(NSUP):
                n0 = s * FREE
                # stacked rhs: partitions 0:48 = raw_hi, 48:96 = raw_lo
                rhs2 = iopool.tile([2 * C * J, FREE], F16, tag="rhs2")
                nc.sync.dma_start(out=rhs2[: C * J], in_=rawh_v[:, n0 : n0 + FREE])
                nc.sync.dma_start(out=rhs2[C * J :], in_=rawl_v[:, n0 : n0 + FREE])

                # f = (Ah+Al)(h+l) ~= Ah*h + [Ah*l + Al*h]  (error ~2^-22)
                fps = pfpool.tile([128, FREE], F32, tag="fps")
                for h in range(NCH):
                    sl = slice(h * 512, (h + 1) * 512)
                    nc.tensor.matmul(fps[:, sl], wfh[:], rhs2[: C * J, sl], start=True, stop=False)
                    nc.tensor.matmul(fps[:, sl], wf2[:], rhs2[:, sl], start=False, stop=True)

                # g = f*inv_dx - mins*inv_dx (fp32, for the DVE-set knots)
                g = gpool.tile([128, FREE], F32, tag="g")
                nc.vector.tensor_scalar(
                    out=g[:],
                    in0=fps[:],
                    scalar1=pT[:, 17:18],
                    scalar2=pT[:, 18:19],
                    op0=mult,
                    op1=add,
                )

                ops = popool.tile([C * J, FREE], F32, tag="ops")
                korder = [0, 16, 12, 13, 14, 15] + list(range(1, 12))
                for ki, k in enumerate(korder):
                    Ck = ccpool.tile([128, FREE], F16, tag="C")
                    if k == 0:
                        # g >= 0 (and a rounding -eps reproduces the
                        # reference's linear extrapolation exactly)
                        nc.vector.tensor_scalar(
                            out=Ck[:], in0=g[:], scalar1=1.0, scalar2=None, op0=mn
                        )
                    elif k == NK - 1:
                        # g < 17 so relu(g-16) < 1: no upper clamp needed
                        nc.vector.tensor_scalar(
                            out=Ck[:],
                            in0=g[:],
                            scalar1=float(k),
                            scalar2=0.0,
                            op0=sub,
                            op1=mx,
                        )
                    else:
                        Rk = rpool.tile([128, FREE], F16, tag="R")
                        if k in ACT_SET:
                            nc.scalar.activation(
                                Rk[:],
                                fps[:],
                                Relu,
                                bias=pT[:, k : k + 1],
                                scale=pT[:, 17:18],
                            )
                        else:
                            nc.vector.tensor_scalar(
                                out=Rk[:],
                                in0=g[:],
                                scalar1=float(k),
                                scalar2=0.0,
                                op0=sub,
                                op1=mx,
                            )
                        nc.vector.tensor_scalar(
                            out=Ck[:], in0=Rk[:], scalar1=1.0, scalar2=None, op0=mn
                        )
                    wk = wks[:, k * C * J : (k + 1) * C * J]
                    for h in range(NCH):
                        nc.tensor.matmul(
                            ops[:, h * 512 : (h + 1) * 512],
                            wk,
                            Ck[:, h * 512 : (h + 1) * 512],
                            start=(ki == 0),
                            stop=(ki == NK - 1),
                        )

                ob = obpool.tile([C * J, FREE], F32, tag="ob")
                nc.vector.tensor_scalar(
                    out=ob[:], in0=ops[:], scalar1=1.0, scalar2=bout[:, 0:1],
                    op0=mult, op1=add,
                )
                nc.sync.dma_start(out=out_v[:, n0 : n0 + FREE], in_=ob[:])
    nc.compile()
    return nc


def _host_params(raw, ys, A):
    in_maps = []
    for b in range(B):
        Ab = A[b].astype(np.float32)
        mins = np.minimum(Ab, 0).sum(axis=0)
        maxs = np.maximum(Ab, 0).sum(axis=0)
        pinv = np.linalg.pinv(Ab).astype(np.float32)  # [8, 3]
        span = (maxs + np.float32(EPS) - mins).astype(np.float32)
        t = np.linspace(0.0, 1.0, K + 2, dtype=np.float32)
        xs = t[None, :] * span[:, None] + mins[:, None]
        dx = (xs[:, 1] - xs[:, 0]).astype(np.float32)
        Y = np.concatenate(
            [mins[:, None], ys[b].astype(np.float32), maxs[:, None]], axis=1
        )  # [8, 18]
        sg = np.diff(Y, axis=1).astype(np.float32)  # [8, 17]
        inv_dx = (np.float32(1.0) / dx).astype(np.float32)

        par = np.zeros((128, 19), np.float32)
        rep = lambda x: np.repeat(x, J, axis=0)
        ks = np.arange(NK, dtype=np.float32)
        par[:, 0:NK] = rep((-(mins * inv_dx))[:, None] - ks[None, :])
        par[:, 17] = np.repeat(inv_dx, J)
        par[:, 18] = np.repeat(-(mins * inv_dx), J)

        wf = np.zeros((C * J, 128), np.float32)
        for j in range(J):
            for c in range(C):
                for a in range(NA):
                    wf[c * J + j, a * J + j] = Ab[c, a]
        wfh = wf.astype(np.float16)
        wfl = (wf - wfh.astype(np.float32)).astype(np.float16)
        wf2 = np.concatenate([wfl, wfh], axis=0)  # rows 0:48 hit hi, 48:96 hit lo

        wks = np.zeros((128, NK * C * J), np.float16)
        for k in range(NK):
            for j in range(J):
                for c in range(C):
                    for a in range(NA):
                        wks[a * J + j, k * C * J + c * J + j] = pinv[a, c] * sg[a, k]

        b0 = (pinv * Y[:, 0:1]).sum(axis=0)  # [3]
        bout = np.repeat(b0[:, None], J, axis=1).reshape(C * J, 1).astype(np.float32)

        rb = np.ascontiguousarray(raw[b].reshape(C, HW), np.float32)
        rh = rb.astype(np.float16)
        rl = (rb - rh.astype(np.float32)).astype(np.float16)
        in_maps.append(
            {
                "rawh": rh,
                "rawl": rl,
                "par": par,
                "wfh": wfh,
                "wf2": wf2,
                "wks": wks,
                "bout": bout,
            }
        )
    return in_maps


def kernel(raw, ys, A):
    raw = np.asarray(raw, np.float32)
    ys = np.asarray(ys, np.float32)
    A = np.asarray(A, np.float32)
    if "nc" not in _NC_CACHE:
        _NC_CACHE["nc"] = _build_nc()
    nc = _NC_CACHE["nc"]
    in_maps = _host_params(raw, ys, A)
    res = run_bass_kernel_spmd(nc, in_maps, core_ids=list(range(B)))
    out = np.stack([res.results[b]["out"].reshape(C, H, W) for b in range(B)])
    return out.astype(np.float32)

